# revision 18
# baseline (speedup 1.0000x reference)
"""Trainium2 Bass kernel for nn_MultiHeadAttention_78134045049371.

Strategy (8 NeuronCores, tensor-parallel over heads):
  - Each core owns H/8 = 2 heads for QKV projection + attention.
  - Host feeds q/k/v in a per-column-block layout ([nt, 128, 4096] fp16,
    8KB contiguous per partition row) plus per-core blocked weight
    slices, so every DMA is descriptor-light and every matmul contracts
    over the partition axis with no on-device transposes.
  - Scores are computed transposed (S^T [keys, q]); softmax over keys
    (= partitions) uses the "ones column" trick: V is augmented with a
    ones column so O_aug = [V|1]^T @ exp(S^T) yields the unnormalized
    output and the exp-sum row in one PSUM accumulation.
  - Normalization happens BEFORE the AllGather: 1/sumexp is computed on
    the core that owns the head (tiny [1,512] ops), broadcast across
    partitions via a DRAM bounce, and multiplied into O^T. The gather
    then ships exactly [128, 512] fp16 per q-block and the fc side needs
    no normalization machinery at all.
  - The gated projection is split by OUTPUT COLUMN: each core computes
    sigmoid(O@Wg^T) * tanh(O@Wfc^T) for its 128 output columns.
  - Emission order keeps the PE dense: attention's kt loop is ACT(exp)-
    bound, so all remaining PE work (other projections, fc blocks) is
    woven INTO the kt loops as small filler pieces, and the tail fc
    blocks are deferred so the last AllGathers are hidden.
  - All matmuls run in fp16 (fp32 accumulation in PSUM); exp/tanh run on
    the ACT engine (one table set).
"""

import sys

for _p in ("/opt/trn_rl_repo", "/root/.axon_site/_ro/trn_rl_repo"):
    if _p not in sys.path:
        sys.path.append(_p)

import numpy as np

import concourse.bass as bass
import concourse.mybir as mybir
import concourse.tile as tile
from concourse import bass_utils
from concourse.vector_clock import ScopedClock

# Problem shape (fixed by the reference)
B, L, D = 2, 2048, 1024
H, DK, DV = 16, 64, 64
NC = 8  # cores
HL = H // NC  # heads per core = 2
BL = B * L  # 4096
TEMP = float(np.sqrt(DK))  # 8.0

NQB = 8  # q-block chunks (also the AllGather pipeline grain)
QB = BL // NQB  # 512 columns per q-block
KT = 128  # key tile (partition dim of S^T)
NKT = L // KT  # 16 key tiles per batch
DCH = D // 128  # 8 contraction chunks of 128
NT_B = L // 512  # 4 column blocks per batch

F16 = mybir.dt.float16
F32 = mybir.dt.float32

MAX_WAITS = 1  # this walrus build encodes at most 1 sem-wait per instruction


def _split_excess_waits(nc):
    """Move excess sem-waits onto NOPs inserted just before the owning
    instruction on the same engine (engine queues are FIFO, so semantics
    are preserved). The walrus build here rejects >1 wait per instruction."""
    for f in nc.m.functions:
        for bb in f.blocks:
            out = []
            changed = False
            for inst in bb.instructions:
                si = inst.sync_info
                waits = list(si.on_wait) if si and si.on_wait else []
                if len(waits) > MAX_WAITS:
                    changed = True
                    k = 0
                    while len(waits) > MAX_WAITS:
                        chunk, waits = waits[:MAX_WAITS], waits[MAX_WAITS:]
                        nop = mybir.InstNoOp(
                            name=f"{inst.name}-wsplit-{k}", ins=[], outs=[]
                        )
                        nop.engine = inst.engine
                        nop.sync_info = mybir.SyncInfo(on_wait=chunk, on_update=[])
                        nc.register_instruction(nop, overwrite=True)
                        out.append(nop)
                        k += 1
                    si.on_wait = waits
                    inst.sync_info = si
                out.append(inst)
            if changed:
                bb.instructions = out


class _TileContext(tile.TileContext):
    """TileContext whose final drain carries its waits on separate NOPs."""

    def _drain_and_barrier(self, tick_clock, wait_clock):
        nc = self.nc
        collector = nc.sync.nop(nofuse=True)
        wait_clock.add_sem_waits(
            collector.ins, ScopedClock({None: tick_clock.global_clock})
        )
        nc.sync.drain()
        nc.all_engine_barrier()
        popped = nc._tile_sem_poison_stack.pop()
        assert popped is self._sem_poison
        nc.clear_and_free_semaphores(list(self.sems.allocated().values()))
        nc.all_engine_barrier()

    def __exit__(self, exc_type, exc_value, traceback):
        super().__exit__(exc_type, exc_value, traceback)
        if exc_type is None:
            _split_excess_waits(self.nc)


def build_kernel():
    nc = bass.Bass(target_bir_lowering=False)

    # Activation streams, blocked per 512-column chunk:
    # xB[nt, p, c*512+w] = x^T[c*128+p, nt*512+w]  (8KB contiguous per row)
    qB = nc.dram_tensor("qB", [NQB, 128, DCH * 512], F16, kind="ExternalInput")
    kB = nc.dram_tensor("kB", [NQB, 128, DCH * 512], F16, kind="ExternalInput")
    vB = nc.dram_tensor("vB", [NQB, 128, DCH * 512], F16, kind="ExternalInput")
    # Blocked weights: w[p, c, m] = W^T[c*128+p, m]
    wqB = nc.dram_tensor("wqB", [128, DCH, HL * DK], F16, kind="ExternalInput")
    wkB = nc.dram_tensor("wkB", [128, DCH, HL * DK], F16, kind="ExternalInput")
    wvB = nc.dram_tensor("wvB", [128, DCH, HL * DV], F16, kind="ExternalInput")
    wfcB = nc.dram_tensor("wfcB", [128, DCH, 128], F16, kind="ExternalInput")
    wgB = nc.dram_tensor("wgB", [128, DCH, 128], F16, kind="ExternalInput")

    # Output: this core's 128 output columns for all B*L rows, transposed.
    out = nc.dram_tensor("out", [128, BL], F16, kind="ExternalOutput")

    # AllGather buffers, NORMALIZED contributions [128, QB] per q-block
    # (2 heads x 64 O^T rows) -> gathered [NC*128, ...] (ranks stack on
    # dim 0). Blocks 0-5 are gathered in PAIRS (halves the per-collective
    # fixed cost on the serial CC queue); blocks 6 and 7 stay single so the
    # tail exposure is one small gather.
    ag_inA = nc.dram_tensor("ag_inA", [3, HL * DV, 2 * QB], F16)
    ag_outA = nc.dram_tensor(
        "ag_outA", [3, NC * HL * DV, 2 * QB], F16, addr_space="Shared"
    )
    ag_inB = nc.dram_tensor("ag_inB", [2, HL * DV, QB], F16)
    ag_outB = nc.dram_tensor(
        "ag_outB", [2, NC * HL * DV, QB], F16, addr_space="Shared"
    )
    # sumexp bounce rows ([2, QB] per q-block): raw sums go out, get re-read
    # spread across 128 partitions (so the iterative reciprocal uses all
    # lanes), and 1/sums comes back for the partition-broadcast read (SBUF
    # sources cannot have partition-step-0 APs, DRAM sources can).
    rbraw = nc.dram_tensor("rbraw", [NQB, HL, QB], F16)
    rb = nc.dram_tensor("rb", [NQB, HL, QB], F16)

    with _TileContext(nc) as tc:
        with (
            tc.tile_pool(name="persist", bufs=1) as persist,
            tc.tile_pool(name="astream", bufs=6) as astream,
            tc.tile_pool(name="exps", bufs=6) as exps,
            tc.tile_pool(name="small", bufs=4) as small,
            tc.tile_pool(name="norm", bufs=2) as normp,
            tc.tile_pool(name="fcin", bufs=3) as fcin,
            tc.tile_pool(name="pp_o", bufs=2, space="PSUM") as pp_o,
            tc.tile_pool(name="pp_fc", bufs=2, space="PSUM") as pp_fc,
            tc.tile_pool(name="pp_s", bufs=2, space="PSUM") as pp_s,
        ):
            # ---- resident tiles ----
            qhTs = [
                persist.tile([HL * DK, QB], F16, name=f"qhT{i}") for i in range(NQB)
            ]
            khTs = [persist.tile([HL * DK, L], F16, name=f"khT{i}") for i in range(B)]
            # vh augmented with a ones column per head: [head][0:64]=vh, [64]=1
            vhs = [
                persist.tile([128, L // 128, HL * (DV + 1)], F16, name=f"vh{i}")
                for i in range(B)
            ]
            wq_sb = persist.tile([128, DCH, HL * DK], F16)
            wk_sb = persist.tile([128, DCH, HL * DK], F16)
            wv_sb = persist.tile([128, DCH, HL * DV], F16)
            wfc_sb = persist.tile([128, DCH, 128], F16)
            wg_sb = persist.tile([128, DCH, 128], F16)

            # weight loads: single clean DMA each (contiguous per partition).
            # k/v weights go first (the lead-in needs them immediately); the
            # fc/gate weights ride the idle gpsimd queue (needed ~80us in).
            nc.sync.dma_start(out=wk_sb[:], in_=wkB[:, :, :])
            nc.sync.dma_start(out=wv_sb[:], in_=wvB[:, :, :])
            nc.sync.dma_start(out=wq_sb[:], in_=wqB[:, :, :])
            nc.gpsimd.dma_start(out=wfc_sb[:], in_=wfcB[:, :, :])
            nc.gpsimd.dma_start(out=wg_sb[:], in_=wgB[:, :, :])

            # ones columns of vh (written once)
            for vh in vhs:
                nc.vector.memset(vh[:, :, DV : DV + 1], 1.0)
                nc.vector.memset(vh[:, :, DV + 1 + DV :], 1.0)

            # ACT table warmup: load the exp/tanh set during the lead-in
            warm = small.tile([128, 1], F32, tag="warm")
            nc.vector.memset(warm[:], 0.0)
            warm2 = small.tile([128, 1], F32, tag="warm")
            nc.scalar.activation(
                out=warm2[:], in_=warm[:], func=mybir.ActivationFunctionType.Exp
            )

            # ================= unit makers =================
            # A "unit" is {dma: closure, pieces: [closures]} where dma issues
            # the unit's input DMA and each piece emits ~2 N=512 matmuls.

            def make_kq_unit(src, wsb, dst_fn, nt, eng=None):
                st = {}

                def dma():
                    xt = astream.tile([128, DCH, 512], F16, tag="xproj", name="xt")
                    (eng or nc.sync).dma_start(
                        out=xt[:],
                        in_=src[nt].rearrange("p (c w) -> p c w", c=DCH),
                    )
                    st["xt"] = xt

                def make_piece(c0):
                    def piece():
                        if c0 == 0:
                            st["ps"] = pp_fc.tile(
                                [128, 512], F32, tag="fcpsum", name="psq"
                            )
                        for c in (c0, c0 + 1):
                            nc.tensor.matmul(
                                st["ps"][:],
                                lhsT=wsb[:, c, :],
                                rhs=st["xt"][:, c, :],
                                start=(c == 0),
                                stop=(c == DCH - 1),
                            )
                        if c0 == DCH - 2:
                            nc.vector.tensor_copy(out=dst_fn(), in_=st["ps"][:])

                    return piece

                return {"dma": dma, "pieces": [make_piece(c) for c in (0, 2, 4, 6)]}

            def make_v_unit(nt, eng=None):
                # one 512-key block: 4 sub-tiles of 128 keys, 8 MMs each
                b = nt // NT_B
                st = {}

                def dma():
                    vt = astream.tile([128, DCH, 512], F16, tag="xproj", name="vt")
                    (eng or nc.sync).dma_start(
                        out=vt[:],
                        in_=vB[nt].rearrange("p (c w) -> p c w", c=DCH),
                    )
                    st["vt"] = vt

                def make_piece(sub):
                    def piece():
                        loc = (nt % NT_B) * 4 + sub
                        ps = pp_fc.tile([128, 512], F32, tag="fcpsum", name="psv")
                        for c in range(DCH):
                            nc.tensor.matmul(
                                ps[:, : HL * DV],
                                lhsT=st["vt"][:, c, bass.ts(sub, 128)],
                                rhs=wv_sb[:, c, :],
                                start=(c == 0),
                                stop=(c == DCH - 1),
                            )
                        for h in range(HL):
                            nc.vector.tensor_copy(
                                out=vhs[b][:, loc, h * (DV + 1) : h * (DV + 1) + DV],
                                in_=ps[:, h * DV : (h + 1) * DV],
                            )

                    return piece

                return {"dma": dma, "pieces": [make_piece(s) for s in range(4)]}

            def make_fc_units(qb):
                # two units: (a) fc matmuls + tanh, (b) gate matmuls + output
                st = {}

                def dma():
                    ot = fcin.tile([128, DCH, QB], F16, tag="fcin", name="ot")
                    if qb < 6:
                        src = ag_outA[qb // 2].rearrange(
                            "(r p) (t w) -> p r t w", p=128, t=2
                        )[:, :, qb % 2, :]
                    else:
                        src = ag_outB[qb - 6].rearrange("(r p) q -> p r q", p=128)
                    nc.gpsimd.dma_start(out=ot[:], in_=src)
                    st["ot"] = ot

                def make_a_piece(c0):
                    def piece():
                        if c0 == 0:
                            st["fps"] = pp_fc.tile(
                                [128, 512], F32, tag="fcpsum", name="fps"
                            )
                        for c in (c0, c0 + 1):
                            nc.tensor.matmul(
                                st["fps"][:],
                                lhsT=wfc_sb[:, c, :],
                                rhs=st["ot"][:, c, :],
                                start=(c == 0),
                                stop=(c == DCH - 1),
                            )
                        if c0 == DCH - 2:
                            tanh_t = small.tile([128, QB], F32, tag="tanh")
                            nc.scalar.activation(
                                out=tanh_t[:],
                                in_=st["fps"][:],
                                func=mybir.ActivationFunctionType.Tanh,
                            )
                            st["tanh"] = tanh_t

                    return piece

                def make_b_piece(c0):
                    def piece():
                        if c0 == 0:
                            st["gps"] = pp_fc.tile(
                                [128, 512], F32, tag="fcpsum", name="gps"
                            )
                        for c in (c0, c0 + 1):
                            nc.tensor.matmul(
                                st["gps"][:],
                                lhsT=wg_sb[:, c, :],
                                rhs=st["ot"][:, c, :],
                                start=(c == 0),
                                stop=(c == DCH - 1),
                            )
                        if c0 == DCH - 2:
                            # sigmoid(g) = 0.5*tanh(g/2) + 0.5 (same table set)
                            sig_t = small.tile([128, QB], F32, tag="sig")
                            nc.scalar.activation(
                                out=sig_t[:],
                                in_=st["gps"][:],
                                func=mybir.ActivationFunctionType.Tanh,
                                scale=0.5,
                            )
                            nc.vector.tensor_scalar(
                                out=sig_t[:],
                                in0=sig_t[:],
                                scalar1=0.5,
                                scalar2=0.5,
                                op0=mybir.AluOpType.mult,
                                op1=mybir.AluOpType.add,
                            )
                            res = small.tile([128, QB], F16, tag="res")
                            nc.vector.tensor_mul(
                                out=res[:], in0=sig_t[:], in1=st["tanh"][:]
                            )
                            nc.gpsimd.dma_start(
                                out=out[:, bass.ts(qb, QB)], in_=res[:]
                            )

                    return piece

                unit_a = {"dma": dma, "pieces": [make_a_piece(c) for c in (0, 2, 4, 6)]}
                unit_b = {
                    "dma": lambda: None,
                    "pieces": [make_b_piece(c) for c in (0, 2, 4, 6)],
                }
                return unit_a, unit_b

            # ================= attention =================
            def attention(qb, units=(), late_units=()):
                """kt loop for q-block qb; `units` DMAs fire at start, their
                pieces weave between kt iterations. `late_units` DMAs fire
                mid-loop (for fc units whose AllGather may still be landing).
                """
                b = qb // (NQB // B)
                for u in units:
                    u["dma"]()
                pieces = [p for u in units for p in u["pieces"]]
                late = list(late_units)

                opsums = [
                    pp_o.tile([DV + 1, QB], F32, tag="opsum", name=f"ops{h}")
                    for h in range(HL)
                ]
                pi = 0
                for kt in range(NKT):
                    if kt == 8:
                        for u in late:
                            u["dma"]()
                            pieces.extend(u["pieces"])
                    sps = pp_s.tile([KT, HL * QB], F32, tag="spsum")
                    for h in range(HL):
                        hp = h * DK
                        nc.tensor.matmul(
                            sps[:, h * QB : (h + 1) * QB],
                            lhsT=khTs[b][hp : hp + DK, kt * KT : (kt + 1) * KT],
                            rhs=qhTs[qb][hp : hp + DK, :],
                            start=True,
                            stop=True,
                        )
                    et = exps.tile([KT, HL * QB], F16, tag="expst")
                    nc.scalar.activation(
                        out=et[:],
                        in_=sps[:],
                        func=mybir.ActivationFunctionType.Exp,
                    )
                    if kt >= 1 and pi < len(pieces):
                        pieces[pi]()
                        pi += 1
                    for h in range(HL):
                        nc.tensor.matmul(
                            opsums[h][:],
                            lhsT=vhs[b][:, kt, h * (DV + 1) : (h + 1) * (DV + 1)],
                            rhs=et[:, h * QB : (h + 1) * QB],
                            start=(kt == 0),
                            stop=(kt == NKT - 1),
                        )
                while pi < len(pieces):
                    pieces[pi]()
                    pi += 1

                # ---- normalize + ship this q-block ----
                # 1) one copy per head evacuates O rows + the exp-sum row
                #    (frees PSUM for the next q-block quickly);
                # 2) sums bounce through DRAM into a [128, 8] spread so the
                #    iterative reciprocal runs on all 128 lanes;
                # 3) 1/sums bounces back, partition-broadcast, multiply.
                oc = []
                for h in range(HL):
                    och = normp.tile([DV + 1, QB], F16, tag=f"oc{h}")
                    nc.vector.tensor_copy(out=och[:], in_=opsums[h][:])
                    oc.append(och)
                for h in range(HL):
                    nc.sync.dma_start(
                        out=rbraw[qb][h : h + 1, :], in_=oc[h][DV : DV + 1, :]
                    )
                rsp = normp.tile([128, (HL * QB) // 128], F16, tag="rsp")
                nc.sync.dma_start(
                    out=rsp[:],
                    in_=rbraw[qb].rearrange("h (p f) -> (h p) f", f=(HL * QB) // 128),
                )
                with nc.allow_low_precision(reason="softmax normalizer in fp16"):
                    nc.vector.reciprocal(out=rsp[:], in_=rsp[:])
                nc.sync.dma_start(
                    out=rb[qb].rearrange("h (p f) -> (h p) f", f=(HL * QB) // 128),
                    in_=rsp[:],
                )
                # partition-broadcast 1/sum across each head's 64 rows.
                # All DVE operands must share base partition 0, so each head
                # gets its own [64, QB] tiles; the two DMAs into ag_in stack
                # the heads on the partition axis.
                for h in range(HL):
                    rbs = normp.tile([DV, QB], F16, tag=f"rbs{h}")
                    nc.sync.dma_start(
                        out=rbs[:],
                        in_=rb[qb, h][None, :].to_broadcast((DV, QB)),
                    )
                    ctile = normp.tile([DV, QB], F16, tag=f"ct{h}")
                    nc.vector.tensor_mul(
                        out=ctile[:], in0=oc[h][:DV, :], in1=rbs[:]
                    )
                    if qb < 6:
                        dst = ag_inA[qb // 2][
                            h * DV : (h + 1) * DV, (qb % 2) * QB : (qb % 2 + 1) * QB
                        ]
                    else:
                        dst = ag_inB[qb - 6][h * DV : (h + 1) * DV, :]
                    nc.sync.dma_start(out=dst, in_=ctile[:])
                if qb in (1, 3, 5):
                    nc.gpsimd.collective_compute(
                        "AllGather",
                        mybir.AluOpType.bypass,
                        replica_groups=[list(range(NC))],
                        ins=[ag_inA[qb // 2]],
                        outs=[ag_outA[qb // 2]],
                    )
                elif qb >= 6:
                    nc.gpsimd.collective_compute(
                        "AllGather",
                        mybir.AluOpType.bypass,
                        replica_groups=[list(range(NC))],
                        ins=[ag_inB[qb - 6]],
                        outs=[ag_outB[qb - 6]],
                    )

            # ================= emission =================
            kq_dst = lambda i: (lambda: qhTs[i][:])
            kh_dst = lambda b, nt: (lambda: khTs[b][:, bass.ts(nt, 512)])

            # lead-in: batch-0 projections, PE-dense; DMAs alternate between
            # the sync and gpsimd queues so two streams fill in parallel
            lead = (
                [
                    make_kq_unit(
                        kB, wk_sb, kh_dst(0, nt), nt,
                        eng=(nc.gpsimd if nt % 2 else nc.sync),
                    )
                    for nt in range(NT_B)
                ]
                + [
                    make_v_unit(nt, eng=(nc.gpsimd if nt % 2 else nc.sync))
                    for nt in range(NT_B)
                ]
                + [make_kq_unit(qB, wq_sb, kq_dst(0), 0)]
            )
            for u in lead[:5]:
                u["dma"]()
            for i, u in enumerate(lead):
                if i + 5 < len(lead):
                    lead[i + 5]["dma"]()
                for p in u["pieces"]:
                    p()

            # filler units for the attention phase
            q0 = [make_kq_unit(qB, wq_sb, kq_dst(i), i) for i in (1, 2, 3)]
            bk = [
                make_kq_unit(kB, wk_sb, kh_dst(1, nt), NT_B + nt)
                for nt in range(NT_B)
            ]
            bv = [make_v_unit(NT_B + nt) for nt in range(NT_B)]
            bq = [make_kq_unit(qB, wq_sb, kq_dst(NT_B + i), NT_B + i) for i in range(4)]
            fc = [make_fc_units(qb) for qb in range(NQB)]  # list of (a, b)

            # The paired gather {qb,qb+1} lands ~(launch skew + chain + AG)
            # after attn(qb+1) ends on this core — about 2 attention widths —
            # so fc_qb is consumed no earlier than attention(qb+4).
            attention(0, units=q0 + [bk[0]])
            attention(1, units=[bk[1], bk[2], bk[3], bv[0]])
            attention(2, units=[bv[1], bv[2], bv[3], bq[0]])
            attention(3, units=[bq[1], bq[2], bq[3]])
            attention(4, units=[fc[0][0], fc[0][1]])
            attention(5, units=[fc[1][0], fc[1][1]])
            attention(6, units=[fc[2][0], fc[2][1]])
            attention(7, units=[fc[3][0], fc[3][1]])

            # tail: the last four fc blocks drain the last AllGathers
            for qb in (4, 5, 6, 7):
                fc[qb][0]["dma"]()
                for p in fc[qb][0]["pieces"]:
                    p()
                for p in fc[qb][1]["pieces"]:
                    p()

    return nc


_NC_CACHE = None


def _get_nc():
    global _NC_CACHE
    if _NC_CACHE is None:
        _NC_CACHE = build_kernel()
    return _NC_CACHE


def _block_stream(xT):
    # xT [D, BL] -> [NQB, 128, DCH*512] with 8KB-contiguous partition rows
    return np.ascontiguousarray(
        xT.reshape(DCH, 128, NQB, 512).transpose(2, 1, 0, 3).reshape(NQB, 128, DCH * 512)
    )


def _block_w(wT):
    # wT [D, m] -> [128, DCH, m]
    m = wT.shape[1]
    return np.ascontiguousarray(wT.reshape(DCH, 128, m).transpose(1, 0, 2))


def prepare_inputs(q, k, v, Wq, bq, Wk, bk, Wv, bv, Wfc, bfc, Wg, bg):
    """Host-side layout prep: transpose + fp16 cast + per-core blocked
    weight slices. Biases are structurally zero and folded out."""
    qT = q.reshape(BL, D).T.astype(np.float16)
    kT = k.reshape(BL, D).T.astype(np.float16)
    vT = v.reshape(BL, D).T.astype(np.float16)
    qBh = _block_stream(qT)
    kBh = _block_stream(kT)
    vBh = _block_stream(vT)
    WqT = (np.asarray(Wq) / TEMP).T.astype(np.float16)  # [D, H*DK], pre-scaled
    WkT = np.asarray(Wk).T.astype(np.float16)
    WvT = np.asarray(Wv).T.astype(np.float16)
    WfcT = np.asarray(Wfc).T.astype(np.float16)  # [H*DV, D]
    WgT = np.asarray(Wg).T.astype(np.float16)

    in_maps = []
    for c in range(NC):
        hs = c * HL * DK
        in_maps.append(
            {
                "qB": qBh,
                "kB": kBh,
                "vB": vBh,
                "wqB": _block_w(WqT[:, hs : hs + HL * DK]),
                "wkB": _block_w(WkT[:, hs : hs + HL * DK]),
                "wvB": _block_w(WvT[:, hs : hs + HL * DV]),
                "wfcB": _block_w(WfcT[:, c * 128 : (c + 1) * 128]),
                "wgB": _block_w(WgT[:, c * 128 : (c + 1) * 128]),
            }
        )
    return in_maps


def assemble_output(results):
    cols = [r["out"] for r in results]  # each [128, BL] fp16 (transposed)
    full = np.concatenate(cols, axis=0)  # [D, BL]
    return np.ascontiguousarray(full.T).reshape(B, L, D).astype(np.float32)


def kernel(**inputs):
    nc = _get_nc()
    in_maps = prepare_inputs(**{k: np.asarray(v) for k, v in inputs.items()})
    res = bass_utils.run_bass_kernel_spmd(nc, in_maps, core_ids=list(range(NC)))
    return assemble_output(res.results)


if __name__ == "__main__":
    nc = build_kernel()
    print("kernel built OK")


# revision 19
# speedup vs baseline: 1.0446x; 1.0446x over previous
"""Trainium2 Bass kernel for nn_MultiHeadAttention_78134045049371.

Strategy (8 NeuronCores, tensor-parallel over heads):
  - Each core owns H/8 = 2 heads for QKV projection + attention.
  - Host feeds q/k/v in a per-column-block layout ([nt, 128, 4096] fp16,
    8KB contiguous per partition row) plus per-core blocked weight
    slices, so every DMA is descriptor-light and every matmul contracts
    over the partition axis with no on-device transposes.
  - Scores are computed transposed (S^T [keys, q]); softmax over keys
    (= partitions) uses the "ones column" trick: V is augmented with a
    ones column so O_aug = [V|1]^T @ exp(S^T) yields the unnormalized
    output and the exp-sum row in one PSUM accumulation.
  - Normalization happens BEFORE the AllGather: 1/sumexp is computed on
    the core that owns the head (tiny [1,512] ops), broadcast across
    partitions via a DRAM bounce, and multiplied into O^T. The gather
    then ships exactly [128, 512] fp16 per q-block and the fc side needs
    no normalization machinery at all.
  - The gated projection is split by OUTPUT COLUMN: each core computes
    sigmoid(O@Wg^T) * tanh(O@Wfc^T) for its 128 output columns.
  - Emission order keeps the PE dense: attention's kt loop is ACT(exp)-
    bound, so all remaining PE work (other projections, fc blocks) is
    woven INTO the kt loops as small filler pieces, and the tail fc
    blocks are deferred so the last AllGathers are hidden.
  - All matmuls run in fp16 (fp32 accumulation in PSUM); exp/tanh run on
    the ACT engine (one table set).
"""

import sys

for _p in ("/opt/trn_rl_repo", "/root/.axon_site/_ro/trn_rl_repo"):
    if _p not in sys.path:
        sys.path.append(_p)

import numpy as np

import concourse.bass as bass
import concourse.mybir as mybir
import concourse.tile as tile
from concourse import bass_utils
from concourse.vector_clock import ScopedClock

# Problem shape (fixed by the reference)
B, L, D = 2, 2048, 1024
H, DK, DV = 16, 64, 64
NC = 8  # cores
HL = H // NC  # heads per core = 2
BL = B * L  # 4096
TEMP = float(np.sqrt(DK))  # 8.0

NQB = 8  # q-block chunks (also the AllGather pipeline grain)
QB = BL // NQB  # 512 columns per q-block
KT = 128  # key tile (partition dim of S^T)
NKT = L // KT  # 16 key tiles per batch
DCH = D // 128  # 8 contraction chunks of 128
NT_B = L // 512  # 4 column blocks per batch

F16 = mybir.dt.float16
F32 = mybir.dt.float32

MAX_WAITS = 1  # this walrus build encodes at most 1 sem-wait per instruction


def _split_excess_waits(nc):
    """Move excess sem-waits onto NOPs inserted just before the owning
    instruction on the same engine (engine queues are FIFO, so semantics
    are preserved). The walrus build here rejects >1 wait per instruction."""
    for f in nc.m.functions:
        for bb in f.blocks:
            out = []
            changed = False
            for inst in bb.instructions:
                si = inst.sync_info
                waits = list(si.on_wait) if si and si.on_wait else []
                if len(waits) > MAX_WAITS:
                    changed = True
                    k = 0
                    while len(waits) > MAX_WAITS:
                        chunk, waits = waits[:MAX_WAITS], waits[MAX_WAITS:]
                        nop = mybir.InstNoOp(
                            name=f"{inst.name}-wsplit-{k}", ins=[], outs=[]
                        )
                        nop.engine = inst.engine
                        nop.sync_info = mybir.SyncInfo(on_wait=chunk, on_update=[])
                        nc.register_instruction(nop, overwrite=True)
                        out.append(nop)
                        k += 1
                    si.on_wait = waits
                    inst.sync_info = si
                out.append(inst)
            if changed:
                bb.instructions = out


class _TileContext(tile.TileContext):
    """TileContext whose final drain carries its waits on separate NOPs."""

    def _drain_and_barrier(self, tick_clock, wait_clock):
        nc = self.nc
        collector = nc.sync.nop(nofuse=True)
        wait_clock.add_sem_waits(
            collector.ins, ScopedClock({None: tick_clock.global_clock})
        )
        nc.sync.drain()
        nc.all_engine_barrier()
        popped = nc._tile_sem_poison_stack.pop()
        assert popped is self._sem_poison
        nc.clear_and_free_semaphores(list(self.sems.allocated().values()))
        nc.all_engine_barrier()

    def __exit__(self, exc_type, exc_value, traceback):
        super().__exit__(exc_type, exc_value, traceback)
        if exc_type is None:
            _split_excess_waits(self.nc)


def build_kernel():
    nc = bass.Bass(target_bir_lowering=False)

    # Activation streams, blocked per 512-column chunk:
    # xB[nt, p, c*512+w] = x^T[c*128+p, nt*512+w]  (8KB contiguous per row)
    qB = nc.dram_tensor("qB", [NQB, 128, DCH * 512], F16, kind="ExternalInput")
    kB = nc.dram_tensor("kB", [NQB, 128, DCH * 512], F16, kind="ExternalInput")
    vB = nc.dram_tensor("vB", [NQB, 128, DCH * 512], F16, kind="ExternalInput")
    # Blocked weights: w[p, c, m] = W^T[c*128+p, m]
    wqB = nc.dram_tensor("wqB", [128, DCH, HL * DK], F16, kind="ExternalInput")
    wkB = nc.dram_tensor("wkB", [128, DCH, HL * DK], F16, kind="ExternalInput")
    wvB = nc.dram_tensor("wvB", [128, DCH, HL * DV], F16, kind="ExternalInput")
    wfcB = nc.dram_tensor("wfcB", [128, DCH, 128], F16, kind="ExternalInput")
    wgB = nc.dram_tensor("wgB", [128, DCH, 128], F16, kind="ExternalInput")

    # Output: this core's 128 output columns for all B*L rows, transposed.
    out = nc.dram_tensor("out", [128, BL], F16, kind="ExternalOutput")

    # AllGather buffers, NORMALIZED contributions [128, QB] per q-block
    # (2 heads x 64 O^T rows) -> gathered [NC*128, ...] (ranks stack on
    # dim 0). Blocks 0-5 are gathered in PAIRS (halves the per-collective
    # fixed cost on the serial CC queue); blocks 6 and 7 stay single so the
    # tail exposure is one small gather.
    ag_inA = nc.dram_tensor("ag_inA", [3, HL * DV, 2 * QB], F16)
    ag_outA = nc.dram_tensor(
        "ag_outA", [3, NC * HL * DV, 2 * QB], F16, addr_space="Shared"
    )
    ag_inB = nc.dram_tensor("ag_inB", [2, HL * DV, QB], F16)
    ag_outB = nc.dram_tensor(
        "ag_outB", [2, NC * HL * DV, QB], F16, addr_space="Shared"
    )
    # sumexp bounce rows ([2, QB] per q-block): raw sums go out, get re-read
    # spread across 128 partitions (so the iterative reciprocal uses all
    # lanes), and 1/sums comes back for the partition-broadcast read (SBUF
    # sources cannot have partition-step-0 APs, DRAM sources can).
    rbraw = nc.dram_tensor("rbraw", [NQB, HL, QB], F16)
    rb = nc.dram_tensor("rb", [NQB, HL, QB], F16)

    with _TileContext(nc) as tc:
        with (
            tc.tile_pool(name="persist", bufs=1) as persist,
            tc.tile_pool(name="astream", bufs=6) as astream,
            tc.tile_pool(name="exps", bufs=6) as exps,
            tc.tile_pool(name="small", bufs=4) as small,
            tc.tile_pool(name="norm", bufs=2) as normp,
            tc.tile_pool(name="fcin", bufs=3) as fcin,
            tc.tile_pool(name="pp_o", bufs=2, space="PSUM") as pp_o,
            tc.tile_pool(name="pp_fc", bufs=2, space="PSUM") as pp_fc,
            tc.tile_pool(name="pp_s", bufs=2, space="PSUM") as pp_s,
        ):
            # ---- resident tiles ----
            qhTs = [
                persist.tile([HL * DK, QB], F16, name=f"qhT{i}") for i in range(NQB)
            ]
            khTs = [persist.tile([HL * DK, L], F16, name=f"khT{i}") for i in range(B)]
            # vh augmented with a ones column per head: [head][0:64]=vh, [64]=1
            vhs = [
                persist.tile([128, L // 128, HL * (DV + 1)], F16, name=f"vh{i}")
                for i in range(B)
            ]
            wq_sb = persist.tile([128, DCH, HL * DK], F16)
            wk_sb = persist.tile([128, DCH, HL * DK], F16)
            wv_sb = persist.tile([128, DCH, HL * DV], F16)
            wfc_sb = persist.tile([128, DCH, 128], F16)
            wg_sb = persist.tile([128, DCH, 128], F16)

            # weight loads: single clean DMA each (contiguous per partition).
            # k/v weights go first (the lead-in needs them immediately); the
            # fc/gate weights ride the idle gpsimd queue (needed ~80us in).
            nc.sync.dma_start(out=wk_sb[:], in_=wkB[:, :, :])
            nc.sync.dma_start(out=wv_sb[:], in_=wvB[:, :, :])
            nc.sync.dma_start(out=wq_sb[:], in_=wqB[:, :, :])
            nc.gpsimd.dma_start(out=wfc_sb[:], in_=wfcB[:, :, :])
            nc.gpsimd.dma_start(out=wg_sb[:], in_=wgB[:, :, :])

            # ones columns of vh (written once)
            for vh in vhs:
                nc.vector.memset(vh[:, :, DV : DV + 1], 1.0)
                nc.vector.memset(vh[:, :, DV + 1 + DV :], 1.0)

            # ACT table warmup: load the exp/tanh set during the lead-in
            warm = small.tile([128, 1], F32, tag="warm")
            nc.vector.memset(warm[:], 0.0)
            warm2 = small.tile([128, 1], F32, tag="warm")
            nc.scalar.activation(
                out=warm2[:], in_=warm[:], func=mybir.ActivationFunctionType.Exp
            )

            # ================= unit makers =================
            # A "unit" is {dma: closure, pieces: [closures]} where dma issues
            # the unit's input DMA and each piece emits ~2 N=512 matmuls.

            def make_kq_unit(src, wsb, dst_fn, nt, eng=None):
                st = {}

                def dma():
                    xt = astream.tile([128, DCH, 512], F16, tag="xproj", name="xt")
                    (eng or nc.sync).dma_start(
                        out=xt[:],
                        in_=src[nt].rearrange("p (c w) -> p c w", c=DCH),
                    )
                    st["xt"] = xt

                def make_piece(c0):
                    def piece():
                        if c0 == 0:
                            st["ps"] = pp_fc.tile(
                                [128, 512], F32, tag="fcpsum", name="psq"
                            )
                        for c in (c0, c0 + 1):
                            nc.tensor.matmul(
                                st["ps"][:],
                                lhsT=wsb[:, c, :],
                                rhs=st["xt"][:, c, :],
                                start=(c == 0),
                                stop=(c == DCH - 1),
                            )
                        if c0 == DCH - 2:
                            nc.vector.tensor_copy(out=dst_fn(), in_=st["ps"][:])

                    return piece

                return {"dma": dma, "pieces": [make_piece(c) for c in (0, 2, 4, 6)]}

            def make_v_unit(nt, eng=None):
                # one 512-key block: 4 sub-tiles of 128 keys, 8 MMs each
                b = nt // NT_B
                st = {}

                def dma():
                    vt = astream.tile([128, DCH, 512], F16, tag="xproj", name="vt")
                    (eng or nc.sync).dma_start(
                        out=vt[:],
                        in_=vB[nt].rearrange("p (c w) -> p c w", c=DCH),
                    )
                    st["vt"] = vt

                def make_piece(sub):
                    def piece():
                        loc = (nt % NT_B) * 4 + sub
                        ps = pp_fc.tile([128, 512], F32, tag="fcpsum", name="psv")
                        for c in range(DCH):
                            nc.tensor.matmul(
                                ps[:, : HL * DV],
                                lhsT=st["vt"][:, c, bass.ts(sub, 128)],
                                rhs=wv_sb[:, c, :],
                                start=(c == 0),
                                stop=(c == DCH - 1),
                            )
                        for h in range(HL):
                            nc.vector.tensor_copy(
                                out=vhs[b][:, loc, h * (DV + 1) : h * (DV + 1) + DV],
                                in_=ps[:, h * DV : (h + 1) * DV],
                            )

                    return piece

                return {"dma": dma, "pieces": [make_piece(s) for s in range(4)]}

            def make_fc_units(qb):
                # two units: (a) fc matmuls + tanh, (b) gate matmuls + output
                st = {}

                def dma():
                    ot = fcin.tile([128, DCH, QB], F16, tag="fcin", name="ot")
                    if qb < 6:
                        src = ag_outA[qb // 2].rearrange(
                            "(r p) (t w) -> p r t w", p=128, t=2
                        )[:, :, qb % 2, :]
                    else:
                        src = ag_outB[qb - 6].rearrange("(r p) q -> p r q", p=128)
                    nc.gpsimd.dma_start(out=ot[:], in_=src)
                    st["ot"] = ot

                def make_a_piece(c0):
                    def piece():
                        if c0 == 0:
                            st["fps"] = pp_fc.tile(
                                [128, 512], F32, tag="fcpsum", name="fps"
                            )
                        for c in (c0, c0 + 1):
                            nc.tensor.matmul(
                                st["fps"][:],
                                lhsT=wfc_sb[:, c, :],
                                rhs=st["ot"][:, c, :],
                                start=(c == 0),
                                stop=(c == DCH - 1),
                            )
                        if c0 == DCH - 2:
                            tanh_t = small.tile([128, QB], F32, tag="tanh")
                            nc.scalar.activation(
                                out=tanh_t[:],
                                in_=st["fps"][:],
                                func=mybir.ActivationFunctionType.Tanh,
                            )
                            st["tanh"] = tanh_t

                    return piece

                def make_b_piece(c0):
                    def piece():
                        if c0 == 0:
                            st["gps"] = pp_fc.tile(
                                [128, 512], F32, tag="fcpsum", name="gps"
                            )
                        for c in (c0, c0 + 1):
                            nc.tensor.matmul(
                                st["gps"][:],
                                lhsT=wg_sb[:, c, :],
                                rhs=st["ot"][:, c, :],
                                start=(c == 0),
                                stop=(c == DCH - 1),
                            )
                        if c0 == DCH - 2:
                            # sigmoid(g) = 0.5*tanh(g/2) + 0.5 (same table set)
                            sig_t = small.tile([128, QB], F32, tag="sig")
                            nc.scalar.activation(
                                out=sig_t[:],
                                in_=st["gps"][:],
                                func=mybir.ActivationFunctionType.Tanh,
                                scale=0.5,
                            )
                            nc.vector.tensor_scalar(
                                out=sig_t[:],
                                in0=sig_t[:],
                                scalar1=0.5,
                                scalar2=0.5,
                                op0=mybir.AluOpType.mult,
                                op1=mybir.AluOpType.add,
                            )
                            res = small.tile([128, QB], F16, tag="res")
                            nc.vector.tensor_mul(
                                out=res[:], in0=sig_t[:], in1=st["tanh"][:]
                            )
                            nc.gpsimd.dma_start(
                                out=out[:, bass.ts(qb, QB)], in_=res[:]
                            )

                    return piece

                unit_a = {"dma": dma, "pieces": [make_a_piece(c) for c in (0, 2, 4, 6)]}
                unit_b = {
                    "dma": lambda: None,
                    "pieces": [make_b_piece(c) for c in (0, 2, 4, 6)],
                }
                return unit_a, unit_b

            # ================= attention =================
            def attention(qb, units=(), late_units=()):
                """kt loop for q-block qb; `units` DMAs fire at start, their
                pieces weave between kt iterations. `late_units` DMAs fire
                mid-loop (for fc units whose AllGather may still be landing).
                """
                b = qb // (NQB // B)
                for u in units:
                    u["dma"]()
                pieces = [p for u in units for p in u["pieces"]]
                late = list(late_units)

                opsums = [
                    pp_o.tile([DV + 1, QB], F32, tag="opsum", name=f"ops{h}")
                    for h in range(HL)
                ]
                pi = 0
                for kt in range(NKT):
                    if kt == 8:
                        for u in late:
                            u["dma"]()
                            pieces.extend(u["pieces"])
                    sps = pp_s.tile([KT, HL * QB], F32, tag="spsum")
                    for h in range(HL):
                        hp = h * DK
                        nc.tensor.matmul(
                            sps[:, h * QB : (h + 1) * QB],
                            lhsT=khTs[b][hp : hp + DK, kt * KT : (kt + 1) * KT],
                            rhs=qhTs[qb][hp : hp + DK, :],
                            start=True,
                            stop=True,
                        )
                    et = exps.tile([KT, HL * QB], F16, tag="expst")
                    nc.scalar.activation(
                        out=et[:],
                        in_=sps[:],
                        func=mybir.ActivationFunctionType.Exp,
                    )
                    if kt >= 1 and pi < len(pieces):
                        pieces[pi]()
                        pi += 1
                    for h in range(HL):
                        nc.tensor.matmul(
                            opsums[h][:],
                            lhsT=vhs[b][:, kt, h * (DV + 1) : (h + 1) * (DV + 1)],
                            rhs=et[:, h * QB : (h + 1) * QB],
                            start=(kt == 0),
                            stop=(kt == NKT - 1),
                        )
                while pi < len(pieces):
                    pieces[pi]()
                    pi += 1

                # ---- normalize + ship this q-block ----
                # 1) one copy per head evacuates O rows + the exp-sum row
                #    (frees PSUM for the next q-block quickly);
                # 2) sums bounce through DRAM into a [128, 8] spread so the
                #    iterative reciprocal runs on all 128 lanes;
                # 3) 1/sums bounces back, partition-broadcast, multiply.
                oc = []
                for h in range(HL):
                    och = normp.tile([DV + 1, QB], F16, tag=f"oc{h}")
                    nc.vector.tensor_copy(out=och[:], in_=opsums[h][:])
                    oc.append(och)
                for h in range(HL):
                    nc.sync.dma_start(
                        out=rbraw[qb][h : h + 1, :], in_=oc[h][DV : DV + 1, :]
                    )
                rsp = normp.tile([128, (HL * QB) // 128], F16, tag="rsp")
                nc.sync.dma_start(
                    out=rsp[:],
                    in_=rbraw[qb].rearrange("h (p f) -> (h p) f", f=(HL * QB) // 128),
                )
                with nc.allow_low_precision(reason="softmax normalizer in fp16"):
                    nc.vector.reciprocal(out=rsp[:], in_=rsp[:])
                nc.sync.dma_start(
                    out=rb[qb].rearrange("h (p f) -> (h p) f", f=(HL * QB) // 128),
                    in_=rsp[:],
                )
                # partition-broadcast 1/sum across each head's 64 rows.
                # All DVE operands must share base partition 0, so each head
                # gets its own [64, QB] tiles; the two DMAs into ag_in stack
                # the heads on the partition axis.
                for h in range(HL):
                    rbs = normp.tile([DV, QB], F16, tag=f"rbs{h}")
                    nc.sync.dma_start(
                        out=rbs[:],
                        in_=rb[qb, h][None, :].to_broadcast((DV, QB)),
                    )
                    ctile = normp.tile([DV, QB], F16, tag=f"ct{h}")
                    nc.vector.tensor_mul(
                        out=ctile[:], in0=oc[h][:DV, :], in1=rbs[:]
                    )
                    if qb < 6:
                        dst = ag_inA[qb // 2][
                            h * DV : (h + 1) * DV, (qb % 2) * QB : (qb % 2 + 1) * QB
                        ]
                    else:
                        dst = ag_inB[qb - 6][h * DV : (h + 1) * DV, :]
                    nc.sync.dma_start(out=dst, in_=ctile[:])
                if qb in (1, 3, 5):
                    nc.gpsimd.collective_compute(
                        "AllGather",
                        mybir.AluOpType.bypass,
                        replica_groups=[list(range(NC))],
                        ins=[ag_inA[qb // 2]],
                        outs=[ag_outA[qb // 2]],
                    )
                elif qb >= 6:
                    nc.gpsimd.collective_compute(
                        "AllGather",
                        mybir.AluOpType.bypass,
                        replica_groups=[list(range(NC))],
                        ins=[ag_inB[qb - 6]],
                        outs=[ag_outB[qb - 6]],
                    )

            # ================= emission =================
            kq_dst = lambda i: (lambda: qhTs[i][:])
            kh_dst = lambda b, nt: (lambda: khTs[b][:, bass.ts(nt, 512)])

            # lead-in: batch-0 projections, PE-dense
            lead = (
                [make_kq_unit(kB, wk_sb, kh_dst(0, nt), nt) for nt in range(NT_B)]
                + [make_v_unit(nt) for nt in range(NT_B)]
                + [make_kq_unit(qB, wq_sb, kq_dst(0), 0)]
            )
            for u in lead[:5]:
                u["dma"]()
            for i, u in enumerate(lead):
                if i + 5 < len(lead):
                    lead[i + 5]["dma"]()
                for p in u["pieces"]:
                    p()

            # filler units for the attention phase
            q0 = [make_kq_unit(qB, wq_sb, kq_dst(i), i) for i in (1, 2, 3)]
            bk = [
                make_kq_unit(kB, wk_sb, kh_dst(1, nt), NT_B + nt)
                for nt in range(NT_B)
            ]
            bv = [make_v_unit(NT_B + nt) for nt in range(NT_B)]
            bq = [make_kq_unit(qB, wq_sb, kq_dst(NT_B + i), NT_B + i) for i in range(4)]
            fc = [make_fc_units(qb) for qb in range(NQB)]  # list of (a, b)

            # The paired gather {qb,qb+1} lands ~(launch skew + chain + AG)
            # after attn(qb+1) ends on this core — about 2 attention widths —
            # so fc_qb is consumed no earlier than attention(qb+4).
            attention(0, units=q0 + [bk[0]])
            attention(1, units=[bk[1], bk[2], bk[3], bv[0]])
            attention(2, units=[bv[1], bv[2], bv[3], bq[0]])
            attention(3, units=[bq[1], bq[2], bq[3]])
            attention(4, units=[fc[0][0], fc[0][1]])
            attention(5, units=[fc[1][0], fc[1][1]])
            attention(6, units=[fc[2][0], fc[2][1]])
            attention(7, units=[fc[3][0], fc[3][1]])

            # tail: the last four fc blocks drain the last AllGathers
            for qb in (4, 5, 6, 7):
                fc[qb][0]["dma"]()
                for p in fc[qb][0]["pieces"]:
                    p()
                for p in fc[qb][1]["pieces"]:
                    p()

    return nc


_NC_CACHE = None


def _get_nc():
    global _NC_CACHE
    if _NC_CACHE is None:
        _NC_CACHE = build_kernel()
    return _NC_CACHE


def _block_stream(xT):
    # xT [D, BL] -> [NQB, 128, DCH*512] with 8KB-contiguous partition rows
    return np.ascontiguousarray(
        xT.reshape(DCH, 128, NQB, 512).transpose(2, 1, 0, 3).reshape(NQB, 128, DCH * 512)
    )


def _block_w(wT):
    # wT [D, m] -> [128, DCH, m]
    m = wT.shape[1]
    return np.ascontiguousarray(wT.reshape(DCH, 128, m).transpose(1, 0, 2))


def prepare_inputs(q, k, v, Wq, bq, Wk, bk, Wv, bv, Wfc, bfc, Wg, bg):
    """Host-side layout prep: transpose + fp16 cast + per-core blocked
    weight slices. Biases are structurally zero and folded out."""
    qT = q.reshape(BL, D).T.astype(np.float16)
    kT = k.reshape(BL, D).T.astype(np.float16)
    vT = v.reshape(BL, D).T.astype(np.float16)
    qBh = _block_stream(qT)
    kBh = _block_stream(kT)
    vBh = _block_stream(vT)
    WqT = (np.asarray(Wq) / TEMP).T.astype(np.float16)  # [D, H*DK], pre-scaled
    WkT = np.asarray(Wk).T.astype(np.float16)
    WvT = np.asarray(Wv).T.astype(np.float16)
    WfcT = np.asarray(Wfc).T.astype(np.float16)  # [H*DV, D]
    WgT = np.asarray(Wg).T.astype(np.float16)

    in_maps = []
    for c in range(NC):
        hs = c * HL * DK
        in_maps.append(
            {
                "qB": qBh,
                "kB": kBh,
                "vB": vBh,
                "wqB": _block_w(WqT[:, hs : hs + HL * DK]),
                "wkB": _block_w(WkT[:, hs : hs + HL * DK]),
                "wvB": _block_w(WvT[:, hs : hs + HL * DV]),
                "wfcB": _block_w(WfcT[:, c * 128 : (c + 1) * 128]),
                "wgB": _block_w(WgT[:, c * 128 : (c + 1) * 128]),
            }
        )
    return in_maps


def assemble_output(results):
    cols = [r["out"] for r in results]  # each [128, BL] fp16 (transposed)
    full = np.concatenate(cols, axis=0)  # [D, BL]
    return np.ascontiguousarray(full.T).reshape(B, L, D).astype(np.float32)


def kernel(**inputs):
    nc = _get_nc()
    in_maps = prepare_inputs(**{k: np.asarray(v) for k, v in inputs.items()})
    res = bass_utils.run_bass_kernel_spmd(nc, in_maps, core_ids=list(range(NC)))
    return assemble_output(res.results)


if __name__ == "__main__":
    nc = build_kernel()
    print("kernel built OK")
